# revision 5
# baseline (speedup 1.0000x reference)
"""Trainium2 Bass kernel for nn_DeepRNN: 3-layer LSTM (B=64,T=512,IN=512,H=1024) + FC(4096).

Strategy: tensor-parallel over the 4H gate dimension across 8 cores (each core
owns a 128-wide H-shard and computes the 4 gates for it), with a per-step
all-gather of the hidden state. The 3 layers run as a wavefront (layer l lags
3l ticks) so their per-step work overlaps. Matmuls in bf16 (fp32 PSUM
accumulate), cell state in fp32.

Per-core layouts:
  - gates psum [128(tok = 2 ticks x 64B), 512] = [i|f|o|g] x 128 cols,
    accumulating input-proj (2-tick batch, M=128) + bias + recurrent matmul
    (per tick, M=64 at row offset (t%2)*64).
  - lhsT for all matmuls = transposed activations hT [128(H-chunk), 64(B)]
    obtained from the all-gather via transposing DMA loads into an SBUF ring.
  - weights resident in SBUF as rhs [128(K-chunk), 512] bf16 tiles.
"""
import os
import numpy as np

import concourse.bass as bass
import concourse.bacc as bacc
import concourse.mybir as mybir
from concourse import tile
from concourse.bass_utils import run_bass_kernel_spmd

try:
    from ml_dtypes import bfloat16 as np_bf16
except ImportError:  # pragma: no cover
    import jax.numpy as jnp
    np_bf16 = jnp.bfloat16


def _ntff_profile_via_ctypes(so_path):
    """(dir, device_ids) -> contextmanager hook driving NTFF profiling via
    ctypes into libaxon_pjrt.so (mirrors trn_agent_boot.trn_boot)."""
    import contextlib
    import ctypes
    import sys

    try:
        lib = ctypes.CDLL(so_path)
    except OSError:
        return None
    if not hasattr(lib, "axon_start_nrt_profile"):
        return None
    lib.axon_start_nrt_profile.argtypes = [
        ctypes.POINTER(ctypes.c_int64),
        ctypes.c_size_t,
    ]
    lib.axon_start_nrt_profile.restype = ctypes.c_int64
    lib.axon_stop_nrt_profile.argtypes = [ctypes.c_char_p]
    lib.axon_stop_nrt_profile.restype = ctypes.c_int64

    @contextlib.contextmanager
    def _hook(output_dir, device_ids):
        import jax

        jax.devices()
        if device_ids:
            ids = (ctypes.c_int64 * len(device_ids))(*device_ids)
            rc = lib.axon_start_nrt_profile(ids, len(device_ids))
        else:
            rc = lib.axon_start_nrt_profile(None, 0)
        if rc != 0:
            raise RuntimeError(f"axon_start_nrt_profile rc={rc}")
        try:
            yield
        finally:
            n = lib.axon_stop_nrt_profile(str(output_dir).encode())
            print(f"profile: {n} file(s) written to {output_dir}",
                  file=sys.stderr)

    return _hook


def _ensure_axon_hooks():
    """run_bass_kernel_spmd(trace=True) imports antenv.axon_hooks; provide a
    registry if the image lacks it, and install the ctypes NTFF hook that the
    boot could not (antenv.axon_hooks was missing at boot time)."""
    import sys
    import types

    try:
        import antenv.axon_hooks as m
    except Exception:
        m = types.ModuleType("antenv.axon_hooks")
        m._h = None
        m.set_axon_ntff_profile_hook = lambda h: setattr(m, "_h", h)
        m.get_axon_ntff_profile_hook = lambda: m._h
        sys.modules["antenv.axon_hooks"] = m
        try:
            import antenv
            antenv.axon_hooks = m
        except Exception:
            pass
    if m.get_axon_ntff_profile_hook() is None:
        hook = _ntff_profile_via_ctypes("/opt/axon/libaxon_pjrt.so")
        if hook is not None:
            m.set_axon_ntff_profile_hook(hook)


_ensure_axon_hooks()


def _patch_sim_unknown_isa():
    """Tile's scheduling simulator has no handler for the cycle-counted NOP
    (ISA opcode 164) used as HAM-keepalive spacing; treat unknown ISA ops as
    zero-cost no-ops during scheduling (hardware executes them natively)."""
    import concourse.bass_interp as _bi
    if getattr(_bi, "_unknown_isa_patched", False):
        return
    _orig = _bi._visit_InstISA

    def _patched(isa, instruction, executor):
        try:
            return _orig(isa, instruction, executor)
        except NotImplementedError:
            return None

    _bi._visit_InstISA = _patched
    _bi._unknown_isa_patched = True


_patch_sim_unknown_isa()

N_CORES = 8
B, IN, H, L, OUT = 64, 512, 1024, 3, 4096
HS = H // N_CORES          # 128  per-core H shard
GS = 4 * HS                # 512  per-core gate shard (i|f|o|g)
LAG = 3                    # wavefront lag between layers
RING = 4                   # hT ring depth (+1 replica slot for wraparound pairs)
F32, BF16 = mybir.dt.float32, mybir.dt.bfloat16
AF = mybir.ActivationFunctionType

_LAST_RESULTS = {}


def _build(nc, T):
    SLOT = 3 * GS  # ring slot free-size (3 layers x [128, 512] bf16)

    xT = nc.dram_tensor("xT", [128, 4 * T * B], BF16, kind="ExternalInput")
    wihT = nc.dram_tensor("wihT", [128, 20 * GS], BF16, kind="ExternalInput")
    whhT = nc.dram_tensor("whhT", [128, 24 * GS], BF16, kind="ExternalInput")
    fcwT = nc.dram_tensor("fcwT", [128, 8 * GS], BF16, kind="ExternalInput")
    # biases: [b0|b1|b2|bfc] each GS wide, then 128 ones
    biases = nc.dram_tensor("biases", [1, 4 * GS + 128], BF16, kind="ExternalInput")
    identd = nc.dram_tensor("ident", [64, 64], BF16, kind="ExternalInput")
    out = nc.dram_tensor("out", [T * B, GS], F32, kind="ExternalOutput")
    debug = os.environ.get("KERNEL_DEBUG", "0") == "1"
    dbg = (nc.dram_tensor("dbg", [1024, 192], F32, kind="ExternalOutput")
           if debug else None)

    xT_v = xT.ap().rearrange("p (k t) -> p k t", k=4)
    xsb_cache = {}

    with tile.TileContext(nc) as tc:
        with (
            tc.tile_pool(name="consts", bufs=1) as cpool,
            tc.tile_pool(name="state", bufs=1) as spool,
            tc.tile_pool(name="xin", bufs=3) as xpool,
            tc.tile_pool(name="tmp", bufs=2) as tpool,
            tc.tile_pool(name="gps", bufs=2, space="PSUM") as gpspool,
            tc.tile_pool(name="fcps", bufs=1, space="PSUM") as fcpool,
            tc.tile_pool(name="trps", bufs=1, space="PSUM") as trpool,
            tc.tile_pool(name="outsb", bufs=2) as opool,
            tc.tile_pool(name="dram", bufs=3, space="DRAM") as dpool,
        ):
            # ---- resident weights / constants ----
            wih_sb = cpool.tile([128, 20 * GS], BF16, name="wih_sb")
            whh_sb = cpool.tile([128, 24 * GS], BF16, name="whh_sb")
            fcw_sb = cpool.tile([128, 8 * GS], BF16, name="fcw_sb")
            bias_sb = cpool.tile([1, 4 * GS + 128], BF16, name="bias_sb")
            nc.sync.dma_start(wih_sb[:], wihT.ap())
            nc.sync.dma_start(whh_sb[:], whhT.ap())
            nc.sync.dma_start(fcw_sb[:], fcwT.ap())
            nc.sync.dma_start(bias_sb[:], biases.ap())
            ident_sb = cpool.tile([64, 64], BF16, name="ident_sb")
            nc.sync.dma_start(ident_sb[:], identd.ap())
            ones_ap = bias_sb[:1, 4 * GS:4 * GS + 128]

            def wih_tile(l, k):  # L0: k=0..3, L1: k=0..7, L2: k=0..7
                base = [0, 4, 12][l] + k
                return wih_sb[:, base * GS:(base + 1) * GS]

            def whh_tile(l, k):
                return whh_sb[:, (8 * l + k) * GS:(8 * l + k + 1) * GS]

            # ---- persistent state ----
            # ringS (slot-major, depth RING): gather-reload dst; all lhsT reads.
            # col = slot*1536 + k*192 + l*64 + b, slot = gather_tick % RING.
            ringS = spool.tile([128, RING * 3 * GS], BF16, name="ringS")
            hT_sb = spool.tile([128, 3 * 64], BF16, name="hT_sb")
            c_st = [[spool.tile([64, HS], F32, name=f"c{l}_{p}") for p in range(2)]
                    for l in range(L)]
            h_all = spool.tile([64, 3 * HS], BF16, name="h_all")
            ifo = [spool.tile([64, 3 * HS], F32, name=f"ifo{l}") for l in range(L)]
            g_t = [spool.tile([64, HS], F32, name=f"g{l}") for l in range(L)]
            tc_t = [spool.tile([64, HS], F32, name=f"tc{l}") for l in range(L)]

            def lhs1(gather_tick, l, k):  # [128, 64] lhsT slice from ringS
                off = (gather_tick % RING) * 1536 + k * 192 + l * 64
                return ringS[:, off:off + 64]

            def prefetch_x(t):
                xsb = xpool.tile([128, 512], BF16, name="xsb")
                nc.sync.dma_start(
                    xsb[:].rearrange("p (k t) -> p k t", k=4),
                    xT_v[:, :, t * 64:(t + 2) * 64])
                xsb_cache[t] = xsb

            gates_ps = {}

            def emit_proj(l, t):
                # 2-tick psum group (t, t+1): input projection + bias
                ps = gpspool.tile([128, GS], F32, name=f"ps{l}", tag=f"ps{l}")
                gates_ps[(l, t // 2)] = ps
                if l == 0:
                    for k in range(4):
                        nc.tensor.matmul(
                            ps[:], xsb_cache[t][:, k * 128:(k + 1) * 128],
                            wih_tile(0, k), start=(k == 0), stop=False)
                else:
                    for half in range(2):
                        r0 = half * 64
                        for k in range(8):
                            nc.tensor.matmul(
                                ps[r0:r0 + 64, :],
                                lhs1(t + half + LAG * (l - 1), l - 1, k),
                                wih_tile(l, k), start=(k == 0), stop=False,
                                tile_position=(0, r0) if r0 else None)
                nc.tensor.matmul(
                    ps[:], ones_ap, bias_sb[:1, l * GS:(l + 1) * GS],
                    start=False, stop=False)

            def emit_fc(tf):
                fps = fcpool.tile([128, GS], F32, name="fps", tag="fps")
                for half in range(2):
                    r0 = half * 64
                    for k in range(8):
                        nc.tensor.matmul(
                            fps[r0:r0 + 64, :],
                            lhs1(tf + half + 2 * LAG, 2, k),
                            fcw_sb[:, k * GS:(k + 1) * GS],
                            start=(k == 0), stop=False,
                            tile_position=(0, r0) if r0 else None)
                nc.tensor.matmul(fps[:], ones_ap, bias_sb[:1, 3 * GS:4 * GS],
                                 start=False, stop=True)
                osb = opool.tile([128, GS], F32, name="osb", tag="osb")
                nc.scalar.copy(osb[:], fps[:])
                nc.scalar.dma_start(out.ap()[tf * 64:(tf + 2) * 64, :], osb[:])

            prefetch_x(0)
            emit_proj(0, 0)

            for s in range(T + 3 * LAG + 2):
                # ---- recurrent matmuls + gate tails ----
                for l in range(L):
                    t = s - LAG * l
                    if not (0 <= t < T):
                        continue
                    ps = gates_ps[(l, t // 2)]
                    r0 = (t % 2) * 64
                    pr = ps[r0:r0 + 64, :]
                    if t > 0:
                        for k in range(8):
                            nc.tensor.matmul(
                                pr, lhs1(s - 1, l, k), whh_tile(l, k),
                                start=False, stop=(k == 7),
                                tile_position=(0, r0) if r0 else None)
                    # c = sig(f)*c + sig(i)*tanh(g); h = sig(o)*tanh(c)
                    nc.scalar.activation(ifo[l][:], pr[:, 0:384], AF.Sigmoid)
                    nc.scalar.activation(g_t[l][:], pr[:, 384:512], AF.Tanh)
                    c_new, c_old = c_st[l][t % 2], c_st[l][1 - t % 2]
                    if t > 0:
                        t1 = tpool.tile([64, HS], F32, name=f"t1{l}", tag=f"t1{l}")
                        t2 = tpool.tile([64, HS], F32, name=f"t2{l}", tag=f"t2{l}")
                        nc.vector.tensor_mul(t1[:], ifo[l][:, 128:256], c_old[:])
                        nc.vector.tensor_mul(t2[:], ifo[l][:, 0:128], g_t[l][:])
                        nc.vector.tensor_add(c_new[:], t1[:], t2[:])
                    else:
                        nc.vector.tensor_mul(c_new[:], ifo[l][:, 0:128], g_t[l][:])
                    nc.scalar.activation(tc_t[l][:], c_new[:], AF.Tanh)
                    nc.vector.tensor_mul(h_all[:, l * HS:(l + 1) * HS],
                                         ifo[l][:, 256:384], tc_t[l][:])

                # ---- transpose h on PE, all-gather hT, reload ring ----
                agdma = None
                if s <= T - 1 + 2 * LAG:
                    trp = trpool.tile([128, 3 * 64], BF16, name="trp", tag="trp")
                    for l in range(L):
                        nc.tensor.transpose(trp[:, l * 64:(l + 1) * 64],
                                            h_all[:, l * HS:(l + 1) * HS],
                                            ident_sb[:])
                    nc.scalar.copy(hT_sb[:], trp[:])
                    agin = dpool.tile([128, 3 * 64], BF16, name="agin")
                    agout = dpool.tile([128 * N_CORES, 3 * 64], BF16,
                                       name="agout", addr_space="Shared")
                    agdma = nc.sync.dma_start(agin[:], hT_sb[:])
                    nc.gpsimd.collective_compute(
                        "AllGather", mybir.AluOpType.bypass,
                        replica_groups=[list(range(N_CORES))],
                        ins=[agin[:]], outs=[agout[:]])
                    if dbg is not None and s == int(os.environ.get("KERNEL_DEBUG_TICK", "0")):
                        nc.gpsimd.dma_start(dbg.ap(), agout[:])
                    slot = s % RING
                    nc.sync.dma_start(
                        ringS[:, slot * 1536:(slot + 1) * 1536]
                        .rearrange("p (c f) -> p c f", c=8),
                        agout[:].rearrange("(c p) f -> p c f", p=128))

                # ---- PE warm-filler during the gather: next tick's proj/FC ----
                tpre = s + 2
                if tpre % 2 == 0 and tpre < T:
                    prefetch_x(tpre)
                for l in range(L):
                    tn = s + 1 - LAG * l
                    if tn % 2 == 0 and 0 <= tn < T and not (l == 0 and tn == 0):
                        emit_proj(l, tn)
                tfn = s + 1 - 3 * LAG
                if tfn % 2 == 0 and 0 <= tfn < T:
                    emit_fc(tfn)

                # ---- HAM keepalive: tiny matmuls through the gather window
                # so the PE clock gate stays at full rate (re-throttles after
                # ~3.4us idle). Chain starts when the agin DMA lands.
                if agdma is not None:
                    warm = trpool.tile([64, 64], F32, name="warm", tag="trp")
                    wk = nc.tensor.nop(nofuse=True)
                    bass._add_dep_helper(wk.ins, agdma.ins, sync=True,
                                         reason="keepalive after agin")
                    prev = wk
                    for j in range(3):
                        mmj = nc.tensor.matmul(
                            warm[:, :], ident_sb[:], ident_sb[:],
                            start=True, stop=True)
                        bass._add_dep_helper(mmj.ins, prev.ins, sync=False,
                                             reason="keepalive order")
                        npj = nc.tensor.nop(cycle_cnt=3500)
                        bass._add_dep_helper(npj.ins, mmj.ins, sync=False,
                                             reason="keepalive spacing")
                        prev = npj

    return nc


def _prep_core_inputs(inputs, core, T):
    """Host-side shard / gate-reorder / transpose for one core."""
    k = core

    def gate_rows(W):  # rows [i | f | o | g] of this core's H-shard; W [4H, ...]
        return np.concatenate(
            [W[0 * H + k * HS:0 * H + (k + 1) * HS],
             W[1 * H + k * HS:1 * H + (k + 1) * HS],
             W[3 * H + k * HS:3 * H + (k + 1) * HS],
             W[2 * H + k * HS:2 * H + (k + 1) * HS]], axis=0)

    def as_ktiles(WT):  # [K, GS] -> [128, (K/128)*GS], K-chunk-major columns
        K = WT.shape[0]
        return np.ascontiguousarray(
            WT.reshape(K // 128, 128, GS).transpose(1, 0, 2).reshape(128, -1))

    wih_parts, whh_parts, bias_parts = [], [], []
    for l in range(L):
        Wg = gate_rows(np.asarray(inputs[f"Wih{l}"], dtype=np.float32))
        wih_parts.append(as_ktiles(np.ascontiguousarray(Wg.T)))
        Hg = gate_rows(np.asarray(inputs[f"Whh{l}"], dtype=np.float32))
        whh_parts.append(as_ktiles(np.ascontiguousarray(Hg.T)))
        b = (np.asarray(inputs[f"bih{l}"], dtype=np.float32)
             + np.asarray(inputs[f"bhh{l}"], dtype=np.float32))
        bias_parts.append(gate_rows(b[:, None])[:, 0])
    fcW = np.asarray(inputs["fcW"], dtype=np.float32)[k * GS:(k + 1) * GS]
    fcb = np.asarray(inputs["fcb"], dtype=np.float32)[k * GS:(k + 1) * GS]

    x = np.asarray(inputs["x"], dtype=np.float32)[:, :T, :]
    xT = np.ascontiguousarray(
        x.transpose(2, 1, 0).reshape(IN, T * B)      # [IN, t*B + b]
        .reshape(4, 128, T * B).transpose(1, 0, 2).reshape(128, 4 * T * B))

    bias_vec = np.concatenate(bias_parts + [fcb, np.ones(128, np.float32)])
    return {
        "ident": np.eye(64, dtype=np.float32).astype(np_bf16),
        "xT": xT.astype(np_bf16),
        "wihT": np.concatenate(wih_parts, axis=1).astype(np_bf16),
        "whhT": np.concatenate(whh_parts, axis=1).astype(np_bf16),
        "fcwT": as_ktiles(np.ascontiguousarray(fcW.T)).astype(np_bf16),
        "biases": bias_vec[None, :].astype(np_bf16),
    }


def kernel(**inputs):
    T = inputs["x"].shape[1]
    nc = bacc.Bacc("TRN2", target_bir_lowering=False, debug=False,
                   num_devices=N_CORES)
    _build(nc, T)
    nc.compile()

    in_maps = [_prep_core_inputs(inputs, c, T) for c in range(N_CORES)]
    trace = os.environ.get("KERNEL_TRACE", "1") == "1"
    res = run_bass_kernel_spmd(nc, in_maps, core_ids=list(range(N_CORES)),
                               trace=trace)
    _LAST_RESULTS["exec_time_ns"] = res.exec_time_ns
    _LAST_RESULTS["res"] = res

    parts = [np.asarray(res.results[c]["out"]) for c in range(N_CORES)]
    full = np.concatenate(parts, axis=1)              # [T*B, 4096], row = t*B+b
    return np.ascontiguousarray(
        full.reshape(T, B, OUT).transpose(1, 0, 2)).astype(np.float32)



# revision 8
# speedup vs baseline: 6.0260x; 6.0260x over previous
"""Trainium2 Bass kernel for nn_DeepRNN: 3-layer LSTM (B=64,T=512,IN=512,H=1024) + FC(4096).

Strategy: tensor-parallel over the 4H gate dimension across 8 cores (each core
owns a 128-wide H-shard and computes the 4 gates for it), with a per-step
all-gather of the hidden state. The 3 layers run as a wavefront (layer l lags
3l ticks) so their per-step work overlaps. Matmuls in bf16 (fp32 PSUM
accumulate), cell state in fp32.

Per-core layouts:
  - gates psum [128(tok = 2 ticks x 64B), 512] = [i|f|o|g] x 128 cols,
    accumulating input-proj (2-tick batch, M=128) + bias + recurrent matmul
    (per tick, M=64 at row offset (t%2)*64).
  - lhsT for all matmuls = transposed activations hT [128(H-chunk), 64(B)]
    obtained from the all-gather via transposing DMA loads into an SBUF ring.
  - weights resident in SBUF as rhs [128(K-chunk), 512] bf16 tiles.
"""
import os
import numpy as np

import concourse.bass as bass
import concourse.bacc as bacc
import concourse.mybir as mybir
from concourse import tile
from concourse.bass_utils import run_bass_kernel_spmd

try:
    from ml_dtypes import bfloat16 as np_bf16
except ImportError:  # pragma: no cover
    import jax.numpy as jnp
    np_bf16 = jnp.bfloat16


def _ntff_profile_via_ctypes(so_path):
    """(dir, device_ids) -> contextmanager hook driving NTFF profiling via
    ctypes into libaxon_pjrt.so (mirrors trn_agent_boot.trn_boot)."""
    import contextlib
    import ctypes
    import sys

    try:
        lib = ctypes.CDLL(so_path)
    except OSError:
        return None
    if not hasattr(lib, "axon_start_nrt_profile"):
        return None
    lib.axon_start_nrt_profile.argtypes = [
        ctypes.POINTER(ctypes.c_int64),
        ctypes.c_size_t,
    ]
    lib.axon_start_nrt_profile.restype = ctypes.c_int64
    lib.axon_stop_nrt_profile.argtypes = [ctypes.c_char_p]
    lib.axon_stop_nrt_profile.restype = ctypes.c_int64

    @contextlib.contextmanager
    def _hook(output_dir, device_ids):
        import jax

        jax.devices()
        if device_ids:
            ids = (ctypes.c_int64 * len(device_ids))(*device_ids)
            rc = lib.axon_start_nrt_profile(ids, len(device_ids))
        else:
            rc = lib.axon_start_nrt_profile(None, 0)
        if rc != 0:
            raise RuntimeError(f"axon_start_nrt_profile rc={rc}")
        try:
            yield
        finally:
            n = lib.axon_stop_nrt_profile(str(output_dir).encode())
            print(f"profile: {n} file(s) written to {output_dir}",
                  file=sys.stderr)

    return _hook


def _ensure_axon_hooks():
    """run_bass_kernel_spmd(trace=True) imports antenv.axon_hooks; provide a
    registry if the image lacks it, and install the ctypes NTFF hook that the
    boot could not (antenv.axon_hooks was missing at boot time)."""
    import sys
    import types

    try:
        import antenv.axon_hooks as m
    except Exception:
        m = types.ModuleType("antenv.axon_hooks")
        m._h = None
        m.set_axon_ntff_profile_hook = lambda h: setattr(m, "_h", h)
        m.get_axon_ntff_profile_hook = lambda: m._h
        sys.modules["antenv.axon_hooks"] = m
        try:
            import antenv
            antenv.axon_hooks = m
        except Exception:
            pass
    if m.get_axon_ntff_profile_hook() is None:
        hook = _ntff_profile_via_ctypes("/opt/axon/libaxon_pjrt.so")
        if hook is not None:
            m.set_axon_ntff_profile_hook(hook)


_ensure_axon_hooks()

N_CORES = 8
B, IN, H, L, OUT = 64, 512, 1024, 3, 4096
HS = H // N_CORES          # 128  per-core H shard
GS = 4 * HS                # 512  per-core gate shard (i|f|o|g)
LAG = 3                    # wavefront lag between layers
RING = 4                   # hT ring depth (+1 replica slot for wraparound pairs)
F32, BF16 = mybir.dt.float32, mybir.dt.bfloat16
AF = mybir.ActivationFunctionType

_LAST_RESULTS = {}


def _build(nc, T):
    SLOT = 3 * GS  # ring slot free-size (3 layers x [128, 512] bf16)

    xT = nc.dram_tensor("xT", [128, 4 * T * B], BF16, kind="ExternalInput")
    wihT = nc.dram_tensor("wihT", [128, 20 * GS], BF16, kind="ExternalInput")
    whhT = nc.dram_tensor("whhT", [128, 24 * GS], BF16, kind="ExternalInput")
    fcwT = nc.dram_tensor("fcwT", [128, 8 * GS], BF16, kind="ExternalInput")
    # biases: [b0|b1|b2|bfc] each GS wide, then 128 ones
    biases = nc.dram_tensor("biases", [1, 4 * GS + 128], BF16, kind="ExternalInput")
    identd = nc.dram_tensor("ident", [64, 64], BF16, kind="ExternalInput")
    out = nc.dram_tensor("out", [T * B, GS], F32, kind="ExternalOutput")
    debug = os.environ.get("KERNEL_DEBUG", "0") == "1"
    dbg = (nc.dram_tensor("dbg", [1024, 192], F32, kind="ExternalOutput")
           if debug else None)

    xT_v = xT.ap().rearrange("p (k t) -> p k t", k=4)
    xsb_cache = {}

    with tile.TileContext(nc) as tc:
        with (
            tc.tile_pool(name="consts", bufs=1) as cpool,
            tc.tile_pool(name="state", bufs=1) as spool,
            tc.tile_pool(name="xin", bufs=3) as xpool,
            tc.tile_pool(name="tmp", bufs=2) as tpool,
            tc.tile_pool(name="gps", bufs=2, space="PSUM") as gpspool,
            tc.tile_pool(name="fcps", bufs=1, space="PSUM") as fcpool,
            tc.tile_pool(name="trps", bufs=1, space="PSUM") as trpool,
            tc.tile_pool(name="outsb", bufs=2) as opool,
            tc.tile_pool(name="dram", bufs=3, space="DRAM") as dpool,
        ):
            # ---- resident weights / constants ----
            wih_sb = cpool.tile([128, 20 * GS], BF16, name="wih_sb")
            whh_sb = cpool.tile([128, 24 * GS], BF16, name="whh_sb")
            fcw_sb = cpool.tile([128, 8 * GS], BF16, name="fcw_sb")
            bias_sb = cpool.tile([1, 4 * GS + 128], BF16, name="bias_sb")
            nc.sync.dma_start(wih_sb[:], wihT.ap())
            nc.sync.dma_start(whh_sb[:], whhT.ap())
            nc.sync.dma_start(fcw_sb[:], fcwT.ap())
            nc.sync.dma_start(bias_sb[:], biases.ap())
            ident_sb = cpool.tile([64, 64], BF16, name="ident_sb")
            nc.sync.dma_start(ident_sb[:], identd.ap())
            ones_ap = bias_sb[:1, 4 * GS:4 * GS + 128]

            def wih_tile(l, k):  # L0: k=0..3, L1: k=0..7, L2: k=0..7
                base = [0, 4, 12][l] + k
                return wih_sb[:, base * GS:(base + 1) * GS]

            def whh_tile(l, k):
                return whh_sb[:, (8 * l + k) * GS:(8 * l + k + 1) * GS]

            # ---- persistent state ----
            # ringS (slot-major, depth RING): gather-reload dst; all lhsT reads.
            # col = slot*1536 + k*192 + l*64 + b, slot = gather_tick % RING.
            ringS = spool.tile([128, RING * 3 * GS], BF16, name="ringS")
            hT_sb = spool.tile([128, 3 * 64], BF16, name="hT_sb")
            c_st = [[spool.tile([64, HS], F32, name=f"c{l}_{p}") for p in range(2)]
                    for l in range(L)]
            h_all = spool.tile([64, 3 * HS], BF16, name="h_all")
            ifo = [spool.tile([64, 3 * HS], F32, name=f"ifo{l}") for l in range(L)]
            g_t = [spool.tile([64, HS], F32, name=f"g{l}") for l in range(L)]
            tc_t = [spool.tile([64, HS], F32, name=f"tc{l}") for l in range(L)]

            def lhs1(gather_tick, l, k):  # [128, 64] lhsT slice from ringS
                # l-major layout: per-layer [128, 8*64] contiguous block so the
                # per-layer all-gather reload lands in one strided DMA.
                off = (gather_tick % RING) * 1536 + l * 512 + k * 64
                return ringS[:, off:off + 64]

            def prefetch_x(t):
                xsb = xpool.tile([128, 512], BF16, name="xsb")
                nc.sync.dma_start(
                    xsb[:].rearrange("p (k t) -> p k t", k=4),
                    xT_v[:, :, t * 64:(t + 2) * 64])
                xsb_cache[t] = xsb

            gates_ps = {}

            def emit_proj(l, t):
                # 2-tick psum group (t, t+1): input projection + bias
                ps = gpspool.tile([128, GS], F32, name=f"ps{l}", tag=f"ps{l}")
                gates_ps[(l, t // 2)] = ps
                if l == 0:
                    for k in range(4):
                        nc.tensor.matmul(
                            ps[:], xsb_cache[t][:, k * 128:(k + 1) * 128],
                            wih_tile(0, k), start=(k == 0), stop=False)
                else:
                    for half in range(2):
                        r0 = half * 64
                        for k in range(8):
                            nc.tensor.matmul(
                                ps[r0:r0 + 64, :],
                                lhs1(t + half + LAG * (l - 1), l - 1, k),
                                wih_tile(l, k), start=(k == 0), stop=False,
                                tile_position=(0, r0) if r0 else None)
                nc.tensor.matmul(
                    ps[:], ones_ap, bias_sb[:1, l * GS:(l + 1) * GS],
                    start=False, stop=False)

            def emit_fc(tf):
                fps = fcpool.tile([128, GS], F32, name="fps", tag="fps")
                for half in range(2):
                    r0 = half * 64
                    for k in range(8):
                        nc.tensor.matmul(
                            fps[r0:r0 + 64, :],
                            lhs1(tf + half + 2 * LAG, 2, k),
                            fcw_sb[:, k * GS:(k + 1) * GS],
                            start=(k == 0), stop=False,
                            tile_position=(0, r0) if r0 else None)
                nc.tensor.matmul(fps[:], ones_ap, bias_sb[:1, 3 * GS:4 * GS],
                                 start=False, stop=True)
                osb = opool.tile([128, GS], F32, name="osb", tag="osb")
                nc.scalar.copy(osb[:], fps[:])
                nc.scalar.dma_start(out.ap()[tf * 64:(tf + 2) * 64, :], osb[:])

            prefetch_x(0)
            emit_proj(0, 0)

            for s in range(T + 3 * LAG + 2):
                # ---- recurrent matmuls + gate tails ----
                for l in range(L):
                    t = s - LAG * l
                    if not (0 <= t < T):
                        continue
                    ps = gates_ps[(l, t // 2)]
                    r0 = (t % 2) * 64
                    pr = ps[r0:r0 + 64, :]
                    if t > 0:
                        for k in range(8):
                            nc.tensor.matmul(
                                pr, lhs1(s - 1, l, k), whh_tile(l, k),
                                start=False, stop=(k == 7),
                                tile_position=(0, r0) if r0 else None)
                    # c = sig(f)*c + sig(i)*tanh(g); h = sig(o)*tanh(c)
                    nc.scalar.activation(ifo[l][:], pr[:, 0:384], AF.Sigmoid)
                    nc.scalar.activation(g_t[l][:], pr[:, 384:512], AF.Tanh)
                    c_new, c_old = c_st[l][t % 2], c_st[l][1 - t % 2]
                    if t > 0:
                        t1 = tpool.tile([64, HS], F32, name=f"t1{l}", tag=f"t1{l}")
                        t2 = tpool.tile([64, HS], F32, name=f"t2{l}", tag=f"t2{l}")
                        nc.vector.tensor_mul(t1[:], ifo[l][:, 128:256], c_old[:])
                        nc.vector.tensor_mul(t2[:], ifo[l][:, 0:128], g_t[l][:])
                        nc.vector.tensor_add(c_new[:], t1[:], t2[:])
                    else:
                        nc.vector.tensor_mul(c_new[:], ifo[l][:, 0:128], g_t[l][:])
                    nc.scalar.activation(tc_t[l][:], c_new[:], AF.Tanh)
                    nc.vector.tensor_mul(h_all[:, l * HS:(l + 1) * HS],
                                         ifo[l][:, 256:384], tc_t[l][:])

                # ---- per-layer: DMA-transpose h, all-gather, reload ring ----
                # One small AllGather per layer, issued as soon as that
                # layer's h is ready, so up to three collectives pipeline
                # across the layer wavefront instead of one serialized 9us
                # combined gather per tick. Reloads ride the scalar HWDGE
                # queue to keep the sync queue for transposes + gather-in.
                if s <= T - 1 + 2 * LAG:
                    slot = s % RING
                    for l in range(L):
                        hT_l = hT_sb[:, l * 64:(l + 1) * 64]
                        nc.sync.dma_start_transpose(
                            hT_l, h_all[:, l * HS:(l + 1) * HS])
                        agin = dpool.tile([128, 64], BF16,
                                          name=f"agin{l}", tag=f"agin{l}")
                        agout = dpool.tile([128 * N_CORES, 64], BF16,
                                           name=f"agout{l}", tag=f"agout{l}",
                                           addr_space="Shared")
                        nc.sync.dma_start(agin[:], hT_l)
                        nc.gpsimd.collective_compute(
                            "AllGather", mybir.AluOpType.bypass,
                            replica_groups=[list(range(N_CORES))],
                            ins=[agin[:]], outs=[agout[:]])
                        nc.scalar.dma_start(
                            ringS[:, slot * 1536 + l * 512:
                                  slot * 1536 + (l + 1) * 512]
                            .rearrange("p (c f) -> p c f", c=8),
                            agout[:].rearrange("(c p) f -> p c f", p=128))

                # ---- PE warm-filler during the gather: next tick's proj/FC ----
                tpre = s + 2
                if tpre % 2 == 0 and tpre < T:
                    prefetch_x(tpre)
                for l in range(L):
                    tn = s + 1 - LAG * l
                    if tn % 2 == 0 and 0 <= tn < T and not (l == 0 and tn == 0):
                        emit_proj(l, tn)
                tfn = s + 1 - 3 * LAG
                if tfn % 2 == 0 and 0 <= tfn < T:
                    emit_fc(tfn)

    return nc


def _prep_core_inputs(inputs, core, T):
    """Host-side shard / gate-reorder / transpose for one core."""
    k = core

    def gate_rows(W):  # rows [i | f | o | g] of this core's H-shard; W [4H, ...]
        return np.concatenate(
            [W[0 * H + k * HS:0 * H + (k + 1) * HS],
             W[1 * H + k * HS:1 * H + (k + 1) * HS],
             W[3 * H + k * HS:3 * H + (k + 1) * HS],
             W[2 * H + k * HS:2 * H + (k + 1) * HS]], axis=0)

    def as_ktiles(WT):  # [K, GS] -> [128, (K/128)*GS], K-chunk-major columns
        K = WT.shape[0]
        return np.ascontiguousarray(
            WT.reshape(K // 128, 128, GS).transpose(1, 0, 2).reshape(128, -1))

    wih_parts, whh_parts, bias_parts = [], [], []
    for l in range(L):
        Wg = gate_rows(np.asarray(inputs[f"Wih{l}"], dtype=np.float32))
        wih_parts.append(as_ktiles(np.ascontiguousarray(Wg.T)))
        Hg = gate_rows(np.asarray(inputs[f"Whh{l}"], dtype=np.float32))
        whh_parts.append(as_ktiles(np.ascontiguousarray(Hg.T)))
        b = (np.asarray(inputs[f"bih{l}"], dtype=np.float32)
             + np.asarray(inputs[f"bhh{l}"], dtype=np.float32))
        bias_parts.append(gate_rows(b[:, None])[:, 0])
    fcW = np.asarray(inputs["fcW"], dtype=np.float32)[k * GS:(k + 1) * GS]
    fcb = np.asarray(inputs["fcb"], dtype=np.float32)[k * GS:(k + 1) * GS]

    x = np.asarray(inputs["x"], dtype=np.float32)[:, :T, :]
    xT = np.ascontiguousarray(
        x.transpose(2, 1, 0).reshape(IN, T * B)      # [IN, t*B + b]
        .reshape(4, 128, T * B).transpose(1, 0, 2).reshape(128, 4 * T * B))

    bias_vec = np.concatenate(bias_parts + [fcb, np.ones(128, np.float32)])
    return {
        "ident": np.eye(64, dtype=np.float32).astype(np_bf16),
        "xT": xT.astype(np_bf16),
        "wihT": np.concatenate(wih_parts, axis=1).astype(np_bf16),
        "whhT": np.concatenate(whh_parts, axis=1).astype(np_bf16),
        "fcwT": as_ktiles(np.ascontiguousarray(fcW.T)).astype(np_bf16),
        "biases": bias_vec[None, :].astype(np_bf16),
    }


def kernel(**inputs):
    T = inputs["x"].shape[1]
    nc = bacc.Bacc("TRN2", target_bir_lowering=False, debug=False,
                   num_devices=N_CORES)
    _build(nc, T)
    nc.compile()

    in_maps = [_prep_core_inputs(inputs, c, T) for c in range(N_CORES)]
    trace = os.environ.get("KERNEL_TRACE", "1") == "1"
    res = run_bass_kernel_spmd(nc, in_maps, core_ids=list(range(N_CORES)),
                               trace=trace)
    _LAST_RESULTS["exec_time_ns"] = res.exec_time_ns
    _LAST_RESULTS["res"] = res

    parts = [np.asarray(res.results[c]["out"]) for c in range(N_CORES)]
    full = np.concatenate(parts, axis=1)              # [T*B, 4096], row = t*B+b
    return np.ascontiguousarray(
        full.reshape(T, B, OUT).transpose(1, 0, 2)).astype(np.float32)

